# revision 11
# baseline (speedup 1.0000x reference)
"""Channel-attention kernel for Trainium2 (8 NeuronCores, data-parallel over batch).

Math: the reference expands x (B,C,T,1) to 8 channels via a 1x1 conv before the
Q@K^T einsum.  Algebraically, with alpha=w1.w2, beta=w1.b2, delta=b1.w2,
eta=b1.b2 and s[b,c]=sum_t x[b,c,t]:

    energy[b,c,e] = alpha*G[b,c,e] + beta*s[b,c] + delta*s[b,e] + T*eta
    G[b] = X[b] @ X[b]^T          (X[b] = x[b,:,:,0], shape (C,T))

The beta*s[c] and T*eta terms are constant along the e (last) axis, so they
cancel in the min-max normalization; only alpha*G + delta*s_e matters.  This
cuts the contraction from T*8 down to T (the advertised 8x headroom).

Per core: 8 batches, processed as 4 pairs of 2 batches stacked on the 128
partitions.  Per pair:
  - load X2 (128,4000) f32, cast to bf16
  - PE-transpose 32 k-tiles of X2bf -> Xt (t on partitions)
  - Gram matmuls (bf16): psum (128,129) accumulates [G2 | s] via a ones column
  - s-row via PE transpose; aux matmul adds (delta/alpha)*s_row to psum
  - extract diagonal (64,64) blocks scaled by alpha, min-max norm + softmax
  - attention (block-diag, scaled by gamma) @ X2bf -> psum, + x, store

Engine-routing note: walrus allows only ONE sync wait on (transpose) matmul
instructions, so every cross-engine producer that PE consumes is kept on DVE
(single semaphore), and two warm-up transposes absorb the identity-DMA waits.
"""

import numpy as np
import ml_dtypes
from contextlib import ExitStack

import concourse.bass as bass
import concourse.tile as tile
from concourse import mybir
from concourse.bass_utils import run_bass_kernel_spmd
from concourse.alu_op_type import AluOpType

F32 = mybir.dt.float32
BF16 = mybir.dt.bfloat16
AX = mybir.AxisListType.X

B, C, T = 64, 64, 4000
NCORES = 8
BPC = B // NCORES          # 8 batches per core
PAIRS = BPC // 2           # 4 pairs of 2 batches
ROWS = BPC * C             # 512 rows of (C,T) per core
NKT = (T + 127) // 128     # 32 k-tiles
TAIL = T - (NKT - 1) * 128  # 32
NCHUNK = 8
CHW = T // NCHUNK          # 500 (fits one PSUM bank in f32)
EPS = 1e-8
TSTRIDE = 129              # per-k-tile stride in Xt (128 data cols + 1 ones col)


def _body(ctx, tc, out_ap, x_ap, idb_ap, idf_ap, alpha, doa, gamma):
    nc = tc.nc
    from bass_rust import add_dep_helper

    # walrus allows a single sync wait on matmul instructions, but PSUM slot
    # reuse needs two (previous PE writer completion + previous reader).  A
    # PE nop with explicit dep edges absorbs both waits; the matmuls that
    # follow (ordered after the nop) then find their ticks already observed.
    def pe_guard(*deps):
        deps = [d for d in deps if d is not None]
        if not deps:
            return None
        n = nc.tensor.nop(hint="dep", nofuse=True)
        for d in deps:
            add_dep_helper(n.ins, d.ins, sync=True, reason="psum slot guard")
        return n

    def order_after(follower, guard):
        if guard is not None:
            add_dep_helper(
                follower.ins, guard.ins, sync=False, reason="order after guard"
            )

    singles = ctx.enter_context(tc.tile_pool(name="singles", bufs=1))
    xfp = ctx.enter_context(tc.tile_pool(name="xfp", bufs=2))
    xbp = ctx.enter_context(tc.tile_pool(name="xbp", bufs=2))
    xtp = ctx.enter_context(tc.tile_pool(name="xtp", bufs=2))
    obp = ctx.enter_context(tc.tile_pool(name="obp", bufs=2))
    attp = ctx.enter_context(tc.tile_pool(name="attp", bufs=2))
    stage = ctx.enter_context(tc.tile_pool(name="stage", bufs=2))
    smalls = ctx.enter_context(tc.tile_pool(name="smalls", bufs=3))

    ps_t = ctx.enter_context(tc.tile_pool(name="ps_t", bufs=3, space="PSUM"))
    ps_s = ctx.enter_context(tc.tile_pool(name="ps_s", bufs=1, space="PSUM"))
    ps_g = ctx.enter_context(tc.tile_pool(name="ps_g", bufs=2, space="PSUM"))
    ps_o = ctx.enter_context(tc.tile_pool(name="ps_o", bufs=2, space="PSUM"))

    ident_bf = singles.tile([128, 128], BF16)
    nc.sync.dma_start(ident_bf[:], idb_ap)
    ident_f32 = singles.tile([128, 128], F32)
    nc.sync.dma_start(ident_f32[:], idf_ap)
    ones_row = singles.tile([1, 128], BF16)
    nc.vector.memset(ones_row[:], 1.0)

    # (writer, reader) history per psum pool tag, for slot-reuse guards
    hist = {"tp": [], "st": [], "g": [], "o": []}

    def slot_guard(tag, bufs):
        i = len(hist[tag])
        if i < bufs:
            return None
        w, r = hist[tag][i - bufs]
        return pe_guard(w, r)

    # PE warm-ups: absorb the identity-DMA waits so real transposes never
    # carry more than the single sync wait walrus allows them.
    warm_b = ps_t.tile([2, 128], BF16, tag="tp")
    wi = nc.tensor.transpose(warm_b[:], ident_bf[:, 0:2], ident_bf[:])
    hist["tp"].append((wi, None))
    warm_f = ps_s.tile([2, 128], F32, tag="st")
    wf = nc.tensor.transpose(warm_f[:], ident_f32[:, 0:2], ident_f32[:])
    hist["st"].append((wf, None))

    for p in range(PAIRS):
        rows = slice(p * 128, (p + 1) * 128)

        x_f32 = xfp.tile([128, T], F32)
        nc.sync.dma_start(x_f32[:], x_ap[rows, :])

        # every PE-consumed producer lives on DVE (single wait semaphore)
        x_bf = xbp.tile([128, T], BF16)
        nc.vector.tensor_copy(x_bf[:], x_f32[:])

        # Transposed copy of x_bf: 32 tiles of (t=128, c=128) at stride 129,
        # with a ones column at offset 128 of each tile (gives s for free).
        xt = xtp.tile([128, NKT * TSTRIDE], BF16)
        xt3 = xt.rearrange("q (k f) -> q k f", f=TSTRIDE)
        nc.vector.memset(xt3[:, :, 128:129], 1.0)

        for g in range(8):  # 8 groups of 4 k-tiles
            grd = slot_guard("tp", 3)
            ps = ps_t.tile([128, 512], BF16, tag="tp")
            last_t = None
            for j in range(4):
                kt = 4 * g + j
                if kt < NKT - 1:
                    last_t = nc.tensor.transpose(
                        ps[:, j * 128:(j + 1) * 128],
                        x_bf[:, kt * 128: kt * 128 + 128],
                        ident_bf[:],
                    )
                else:
                    last_t = nc.tensor.transpose(
                        ps[0:TAIL, j * 128:(j + 1) * 128],
                        x_bf[:, kt * 128: kt * 128 + TAIL],
                        ident_bf[:],
                    )
                order_after(last_t, grd)
            if g < 7:
                cp = nc.vector.tensor_copy(
                    xt3[:, 4 * g: 4 * g + 4, 0:128],
                    ps.rearrange("q (j f) -> q j f", f=128),
                )
            else:
                nc.vector.tensor_copy(
                    xt3[:, 28:31, 0:128],
                    ps[:, 0:384].rearrange("q (j f) -> q j f", f=128),
                )
                cp = nc.vector.tensor_copy(
                    xt3[0:TAIL, 31:32, 0:128],
                    ps[0:TAIL, 384:512].rearrange("q (j f) -> q j f", f=128),
                )
            hist["tp"].append((last_t, cp))

        # Gram accumulation: psum_g[:, 0:128] = G2, psum_g[:, 128] = s
        grd = slot_guard("g", 2)
        psum_g = ps_g.tile([128, 129], F32, tag="g")
        for kt in range(NKT):
            kk = 128 if kt < NKT - 1 else TAIL
            base = kt * TSTRIDE
            mm = nc.tensor.matmul(
                psum_g[:],
                lhsT=xt[0:kk, base: base + 128],
                rhs=xt[0:kk, base: base + 129],
                start=(kt == 0),
                stop=(kt == NKT - 1),
            )
            order_after(mm, grd)

        # s as a row; aux matmul adds (delta/alpha) * s_row to every row of G2
        s_col = smalls.tile([128, 1], F32)
        nc.vector.tensor_copy(s_col[:], psum_g[:, 128:129])
        grd = slot_guard("st", 1)
        st_ps = ps_s.tile([1, 128], F32, tag="st")
        stt = nc.tensor.transpose(st_ps[:], s_col[:], ident_f32[:])
        order_after(stt, grd)
        rhs_aux = smalls.tile([1, 128], BF16)
        rax = nc.vector.tensor_scalar_mul(rhs_aux[:], st_ps[:], doa)
        hist["st"].append((stt, rax))
        aux = nc.tensor.matmul(
            psum_g[:, 0:128],
            lhsT=ones_row[:],
            rhs=rhs_aux[:],
            start=False,
            stop=True,
            skip_group_check=True,
        )

        # Diagonal (64,64) blocks, scaled by alpha -> energy (128, 64)
        e_sb = smalls.tile([128, 64], F32)
        nc.vector.tensor_scalar_mul(e_sb[0:64, :], psum_g[0:64, 0:64], alpha)
        er = nc.vector.tensor_scalar_mul(
            e_sb[64:128, :], psum_g[64:128, 64:128], alpha
        )
        hist["g"].append((aux, er))

        # min-max normalize along free axis, then softmax (normalized values
        # live in [0,1], so no max-subtraction is needed before exp)
        rmax = smalls.tile([128, 1], F32)
        nc.vector.tensor_reduce(rmax[:], e_sb[:], axis=AX, op=AluOpType.max)
        rmin = smalls.tile([128, 1], F32)
        nc.vector.tensor_reduce(rmin[:], e_sb[:], axis=AX, op=AluOpType.min)
        den = smalls.tile([128, 1], F32)
        nc.vector.tensor_scalar(
            den[:], rmax[:], scalar1=rmin[:], scalar2=EPS,
            op0=AluOpType.subtract, op1=AluOpType.add,
        )
        rden = smalls.tile([128, 1], F32)
        nc.vector.reciprocal(rden[:], den[:])
        nbias = smalls.tile([128, 1], F32)
        nc.vector.scalar_tensor_tensor(
            nbias[:], in0=rmin[:], scalar=-1.0, in1=rden[:],
            op0=AluOpType.mult, op1=AluOpType.mult,
        )
        ex = smalls.tile([128, 64], F32)
        nc.scalar.activation(
            ex[:], e_sb[:], mybir.ActivationFunctionType.Exp,
            bias=nbias[:], scale=rden[:],
        )
        ssum = smalls.tile([128, 1], F32)
        nc.vector.tensor_reduce(ssum[:], ex[:], axis=AX, op=AluOpType.add)
        rsum = smalls.tile([128, 1], F32)
        nc.vector.reciprocal(rsum[:], ssum[:])

        # Block-diagonal attention, scaled by gamma (folded into softmax norm)
        latt = attp.tile([128, 128], BF16)
        nc.vector.memset(latt[:], 0.0)
        nc.vector.tensor_scalar(
            latt[0:64, 0:64], ex[0:64, :], scalar1=rsum[0:64], scalar2=gamma,
            op0=AluOpType.mult, op1=AluOpType.mult,
        )
        nc.vector.tensor_scalar(
            latt[64:128, 64:128], ex[64:128, :], scalar1=rsum[64:128],
            scalar2=gamma, op0=AluOpType.mult, op1=AluOpType.mult,
        )

        # attended chunks + residual add + store.  Even chunks: DVE add from
        # PSUM.  Odd chunks: ACT copies PSUM->SBUF, GPSIMD does the add —
        # keeps DVE from becoming the bottleneck.  psum_o has bufs=2 so each
        # slot's previous reader stays on a single engine (1-wait matmuls).
        out_sb = obp.tile([128, T], F32)
        att_st = stage.tile([128, 4, CHW], F32)
        for ch in range(NCHUNK):
            cols = slice(ch * CHW, (ch + 1) * CHW)
            grd = slot_guard("o", 2)
            psum_o = ps_o.tile([128, CHW], F32, tag="o")
            mm = nc.tensor.matmul(
                psum_o[:], lhsT=latt[:], rhs=x_bf[:, cols], start=True, stop=True
            )
            order_after(mm, grd)
            if ch % 2 == 0:
                rd = nc.vector.tensor_add(
                    out_sb[:, cols], psum_o[:], x_f32[:, cols]
                )
            else:
                rd = nc.scalar.copy(att_st[:, ch // 2, :], psum_o[:])
            hist["o"].append((mm, rd))
        oddv = out_sb.rearrange("q (c w) -> q c w", w=CHW)[:, 1::2, :]
        xodd = x_f32.rearrange("q (c w) -> q c w", w=CHW)[:, 1::2, :]
        nc.gpsimd.tensor_add(oddv, att_st[:], xodd)
        nc.sync.dma_start(out_ap[rows, :], out_sb[:])


_MODULE_CACHE = {}


def _build_module(alpha, doa, gamma):
    key = (alpha, doa, gamma)
    if key in _MODULE_CACHE:
        return _MODULE_CACHE[key]
    nc = bass.Bass(
        "TRN2", target_bir_lowering=False, debug=False, num_devices=NCORES
    )
    x_ap = nc.dram_tensor("x", (ROWS, T), F32, kind="ExternalInput").ap()
    idb_ap = nc.dram_tensor("idb", (128, 128), BF16, kind="ExternalInput").ap()
    idf_ap = nc.dram_tensor("idf", (128, 128), F32, kind="ExternalInput").ap()
    out_ap = nc.dram_tensor("out", (ROWS, T), F32, kind="ExternalOutput").ap()
    with tile.TileContext(nc) as tc, ExitStack() as ctx:
        _body(ctx, tc, out_ap, x_ap, idb_ap, idf_ap, alpha, doa, gamma)
    _split_mm_waits(nc)
    _MODULE_CACHE[key] = nc
    return nc


_WAIT_EXEMPT = {"InstNoOp"}


def _split_mm_waits(nc):
    """walrus TRN2 codegen allows only ONE sync wait on compute instructions
    (Matmult, TensorScalar, ...).  When Tile emits more (e.g. PSUM slot
    reuse: previous-writer completion + previous-reader), hoist the extras
    onto same-engine NoOps inserted immediately before — the sequencer
    dispatches in order, so the blocking semantics are identical."""
    nid = [0]
    for f in nc.m.functions:
        for block in f.blocks:
            out = []
            for inst in block.instructions:
                si = getattr(inst, "sync_info", None)
                if (
                    si is not None
                    and si.on_wait
                    and len(si.on_wait) > 1
                    and type(inst).__name__ not in _WAIT_EXEMPT
                ):
                    waits = list(si.on_wait)
                    for w in waits[:-1]:
                        nid[0] += 1
                        out.append(
                            mybir.InstNoOp(
                                name=f"{inst.name}-wsplit{nid[0]}",
                                engine=inst.engine,
                                ins=[],
                                outs=[],
                                sync_info=mybir.SyncInfo(
                                    on_wait=[w], on_update=[]
                                ),
                                text_hint="wait-split",
                                bass_nofuse=True,
                            )
                        )
                    inst.sync_info = mybir.SyncInfo(
                        on_wait=waits[-1:], on_update=list(si.on_update)
                    )
                out.append(inst)
            block.instructions[:] = out


def _prepare(inputs):
    x = np.ascontiguousarray(
        np.asarray(inputs["x"], dtype=np.float32).reshape(B * C, T)
    )
    w1 = np.asarray(inputs["w1"], dtype=np.float64)
    b1 = np.asarray(inputs["b1"], dtype=np.float64)
    w2 = np.asarray(inputs["w2"], dtype=np.float64)
    b2 = np.asarray(inputs["b2"], dtype=np.float64)
    gamma = float(np.asarray(inputs["gamma"]))
    alpha = float(w1 @ w2)
    delta = float(b1 @ w2)
    assert abs(alpha) > 1e-12, "degenerate alpha not supported"
    nc = _build_module(alpha, delta / alpha, gamma)
    ident_b = np.eye(128, dtype=ml_dtypes.bfloat16)
    ident_f = np.eye(128, dtype=np.float32)
    in_maps = [
        {
            "x": x[i * ROWS:(i + 1) * ROWS],
            "idb": ident_b,
            "idf": ident_f,
        }
        for i in range(NCORES)
    ]
    return nc, in_maps


def kernel(**inputs):
    nc, in_maps = _prepare(inputs)
    res = run_bass_kernel_spmd(nc, in_maps, core_ids=list(range(NCORES)))
    out = np.concatenate([res.results[i]["out"] for i in range(NCORES)], axis=0)
    return out.reshape(B, C, T, 1)
